# revision 19
# baseline (speedup 1.0000x reference)
"""Trainium2 Bass kernel for the BSDE solver (nn_BSDE_solver).

Data-parallel over Monte-Carlo paths: batch B=8192 sharded across 8
NeuronCores (1024 paths/core); MLP params, volatility matrix and
per-step scalars replicated. The 49-step time scan runs fully
unrolled on each core.

Layout: activations are feature-major ([feature, batch]) so every MLP
layer is one matmul group with lhsT = the weight matrix as stored
([in, out]) and rhs = activations^T. The time coordinate of the MLP
inputs is folded into a per-step bias (b + t*W[0,:]) on the host;
sqrt(dt) is folded into the volatility matrix per step on the host.

The two MLP evaluations per step are independent given S; the v-net
evaluation is offset by one step (v-eval(s) computes vOld_s at (t_s,
S_s)) and interleaved with g-MLP(s) at m-tile granularity so the PE
never waits on PSUM->SBUF evacuations.

Matmuls run in float32r (single-pass reduced-precision fp32, ~tf32):
all matmul operands must be produced by rounding ops, so weights/dW
are cast-DMA'd via gpsimd and activations are written as float32r by
the evacuation instructions. The S state trajectory stays full fp32.
"""

import numpy as np

# problem constants (hardcoded per spec)
B, DIM, N, W, L = 8192, 100, 50, 256, 4
R = 0.05
NCORES = 8
BC = B // NCORES      # paths per core
CH = 512              # free-dim chunk = one PSUM bank of fp32
NCH = BC // CH
NS = N - 1            # time steps
KT = W // 128         # k-tiles per hidden matmul
MT = W // 128         # m-tiles per hidden layer
NH = L - 1            # hidden layers

MM_DTYPE_NAME = "float32r"

_cache = {}


def _build(one_prh, bvo, mm_name, ns):
    import concourse.bacc as bacc
    import concourse.mybir as mybir
    import concourse.tile as tile
    from contextlib import ExitStack

    F32 = mybir.dt.float32
    MMDT = getattr(mybir.dt, mm_name)
    AF = mybir.ActivationFunctionType
    ALU = mybir.AluOpType
    rounded = MMDT != F32

    nc = bacc.Bacc("TRN2", target_bir_lowering=False, debug=False)

    def din(name, shape):
        return nc.dram_tensor(name, shape, F32, kind="ExternalInput").ap()

    def dout(name, shape):
        return nc.dram_tensor(name, shape, F32, kind="ExternalOutput").ap()

    S0T = din("S0T", [DIM, BC])
    DWT = din("DWT", [NS, DIM, BC])
    VT = din("VT", [DIM, NS * DIM])
    WG1 = din("WG1", [DIM, W])
    WGH = din("WGH", [128, NH * KT * W])
    WGO = din("WGO", [128, KT * DIM])
    WV1 = din("WV1", [DIM, W])
    WVH = din("WVH", [128, NH * KT * W])
    WVO = din("WVO", [128, KT * 1])
    BG = din("BG", [128, MT * NS])
    BV = din("BV", [128, MT * (NS + 1)])
    BGH = din("BGH", [128, MT * NH])
    BVH = din("BVH", [128, MT * NH])
    BGO = din("BGO", [DIM, 1])
    ERS = din("ERS", [NS, 1])
    SFT = dout("SFT", [DIM, BC])
    VF = dout("VF", [1, BC])
    ERR = dout("ERR", [1, BC])

    with tile.TileContext(nc) as tc, ExitStack() as ctx:
        const = ctx.enter_context(tc.tile_pool(name="const", bufs=1))
        dwp = ctx.enter_context(tc.tile_pool(name="dw", bufs=3))
        sp = ctx.enter_context(tc.tile_pool(name="sp", bufs=3))
        smmp = ctx.enter_context(tc.tile_pool(name="smm", bufs=3))
        hp = ctx.enter_context(tc.tile_pool(name="hp", bufs=5))
        gp = ctx.enter_context(tc.tile_pool(name="gp", bufs=2))
        volp = ctx.enter_context(tc.tile_pool(name="volp", bufs=3))
        ps_g = ctx.enter_context(tc.tile_pool(name="psg", bufs=2, space="PSUM"))
        ps_v = ctx.enter_context(tc.tile_pool(name="psv", bufs=2, space="PSUM"))

        def ctile(shape, src, tag, dt=F32):
            t = const.tile(shape, dt, tag=tag)
            if dt == F32:
                nc.sync.dma_start(t[:], src)
            else:
                nc.gpsimd.dma_start(t[:], src)
            return t

        vt_sb = const.tile([DIM, NS * DIM], MMDT, tag="vt")
        nc.gpsimd.dma_start(vt_sb[:, 0:DIM], VT[:, 0:DIM])
        wv1_sb = ctile([DIM, W], WV1[:], "wv1", MMDT)
        wg1_sb = ctile([DIM, W], WG1[:], "wg1", MMDT)
        wvh_sb = ctile([128, NH * KT * W], WVH[:], "wvh", MMDT)
        wgh_sb = ctile([128, NH * KT * W], WGH[:], "wgh", MMDT)
        wvo_sb = ctile([128, KT * 1], WVO[:], "wvo", MMDT)
        wgo_sb = ctile([128, KT * DIM], WGO[:], "wgo", MMDT)
        nc.gpsimd.dma_start(vt_sb[:, DIM:], VT[:, DIM:])
        bg_sb = ctile([128, MT * NS], BG[:], "bg")
        bv_sb = ctile([128, MT * (NS + 1)], BV[:], "bv")
        bgh_sb = ctile([128, MT * NH], BGH[:], "bgh")
        bvh_sb = ctile([128, MT * NH], BVH[:], "bvh")
        bgo_sb = ctile([DIM, 1], BGO[:], "bgo")
        ers_sb = ctile([NS, 1], ERS[:], "ers")

        ones_f = const.tile([DIM, 1], F32, tag="ones_f")
        nc.gpsimd.memset(ones_f[:], 1.0)
        ones49_f = const.tile([NS, 1], F32, tag="o49_f")
        nc.gpsimd.memset(ones49_f[:], 1.0)
        if rounded:
            ones_sb = const.tile([DIM, 1], MMDT, tag="ones")
            nc.gpsimd.tensor_copy(ones_sb[:], ones_f[:])
            ones49 = const.tile([NS, 1], MMDT, tag="o49")
            nc.gpsimd.tensor_copy(ones49[:], ones49_f[:])
        else:
            ones_sb, ones49 = ones_f, ones49_f

        vold = const.tile([NS, BC], F32, tag="vold")
        vnew = const.tile([NS, BC], F32, tag="vnew")
        stbuf = const.tile([NS, BC], F32, tag="stbuf")

        def evac(dst, src, bias, relu, eng):
            if relu:
                # split across both engines: halves the critical latency
                h = CH
                nc.scalar.activation(dst[:, :h], src[:, :h], AF.Relu, bias=bias)
                nc.vector.tensor_scalar(dst[:, h:], src[:, h:], bias, 0.0,
                                        ALU.add, ALU.max)
            elif eng == "act":
                if isinstance(bias, float):
                    nc.scalar.activation(dst, src, AF.Copy, bias=bias)
                else:
                    nc.scalar.activation(dst, src, AF.Identity, bias=bias)
            else:
                nc.vector.tensor_scalar(dst, src, bias, None, ALU.add)

        ENG = {0: "dve", 1: "act"}

        def mlp_core_gen(x_mm, w1_sb, wh_sb, b1col, b1_sb, b1_ncols, bh_sb,
                         pool, ptag, out):
            """Yields after each (layer, m-tile) phase; leaves the final
            hidden tiles in out[0:2]."""
            h_prev = [None, None]
            for mt in range(MT):
                ps = pool.tile([128, BC], F32, tag=ptag)
                for c in range(NCH):
                    nc.tensor.matmul(
                        ps[:, c * CH:(c + 1) * CH],
                        w1_sb[:, mt * 128:(mt + 1) * 128],
                        x_mm[:, c * CH:(c + 1) * CH],
                        start=True, stop=True,
                    )
                h = hp.tile([128, BC], MMDT, tag=ptag + "h")
                evac(h[:], ps[:],
                     b1_sb[:, mt * b1_ncols + b1col:mt * b1_ncols + b1col + 1],
                     True, ENG[mt])
                h_prev[mt] = h
                yield
            for l in range(NH):
                h_new = [None, None]
                for mt in range(MT):
                    ps = pool.tile([128, BC], F32, tag=ptag)
                    for kt in range(KT):
                        wsl = wh_sb[:, (l * KT + kt) * W + mt * 128:
                                    (l * KT + kt) * W + (mt + 1) * 128]
                        for c in range(NCH):
                            nc.tensor.matmul(
                                ps[:, c * CH:(c + 1) * CH],
                                wsl,
                                h_prev[kt][:, c * CH:(c + 1) * CH],
                                start=(kt == 0), stop=(kt == KT - 1),
                            )
                    h = hp.tile([128, BC], MMDT, tag=ptag + "h")
                    evac(h[:], ps[:], bh_sb[:, mt * NH + l:mt * NH + l + 1],
                         True, ENG[mt])
                    h_new[mt] = h
                    yield
                h_prev = h_new
            out[0], out[1] = h_prev

        def gen_g(x_mm, s, out_grad):
            h = [None, None]
            yield from mlp_core_gen(x_mm, wg1_sb, wgh_sb, s, bg_sb, NS,
                                    bgh_sb, ps_g, "pg", h)
            ps = ps_g.tile([128, BC], F32, tag="pg")
            for kt in range(KT):
                for c in range(NCH):
                    nc.tensor.matmul(
                        ps[:DIM, c * CH:(c + 1) * CH],
                        wgo_sb[:, kt * DIM:(kt + 1) * DIM],
                        h[kt][:, c * CH:(c + 1) * CH],
                        start=(kt == 0), stop=(kt == KT - 1),
                    )
            grad = gp.tile([DIM, BC], F32, tag="grad")
            evac(grad[:], ps[:DIM, :], bgo_sb[:, 0:1], False, "act")
            out_grad[0] = grad
            yield

        def gen_v(x_mm, s):
            """v-eval at (t_s, S_s): writes vold[s], vnew[s-1], VF at the end."""
            h = [None, None]
            yield from mlp_core_gen(x_mm, wv1_sb, wvh_sb, s, bv_sb, NS + 1,
                                    bvh_sb, ps_v, "pv", h)
            ps = ps_v.tile([128, BC], F32, tag="pv")
            for kt in range(KT):
                for c in range(NCH):
                    nc.tensor.matmul(
                        ps[:1, c * CH:(c + 1) * CH],
                        wvo_sb[:, kt:kt + 1],
                        h[kt][:, c * CH:(c + 1) * CH],
                        start=(kt == 0), stop=(kt == KT - 1),
                    )
            vtmp = gp.tile([1, BC], F32, tag="vtmp")
            evac(vtmp[:], ps[:1, :], float(bvo), False, "act")
            if s <= ns - 1:
                nc.sync.dma_start(vold[s:s + 1, :], vtmp[:])
            if s >= 1:
                nc.sync.dma_start(vnew[s - 1:s, :], vtmp[:])
            if s == ns:
                nc.sync.dma_start(VF[:], vtmp[:])
            yield

        def roundcpy(t):
            if not rounded:
                return t
            r = smmp.tile([DIM, BC], MMDT, tag="smm")
            nc.gpsimd.dma_start(r[:], t[:])
            return r

        def emit_stoch(pprod, s):
            pss = ps_g.tile([128, BC], F32, tag="pg")
            for c in range(NCH):
                nc.tensor.matmul(
                    pss[:1, c * CH:(c + 1) * CH],
                    ones_sb[:, 0:1],
                    pprod[:, c * CH:(c + 1) * CH],
                    start=True, stop=True,
                )
            stmp = gp.tile([1, BC], F32, tag="stmp")
            evac(stmp[:], pss[:1, :], 0.0, False, "act")
            nc.sync.dma_start(stbuf[s:s + 1, :], stmp[:])

        s_S = [None] * (ns + 1)   # S_k full-precision state tiles
        s_M = [None] * (ns + 1)   # rounded copies for matmul inputs
        vol_t = [None] * ns

        s_S[0] = sp.tile([DIM, BC], F32, tag="sp", name="s0")
        nc.sync.dma_start(s_S[0][:], S0T[:])
        s_M[0] = roundcpy(s_S[0])

        def chain(k):
            # S_k -> S_{k+1}: dW load, vol matmul, vol product, state update
            dwt = dwp.tile([DIM, BC], MMDT, tag="dw")
            if rounded:
                nc.gpsimd.dma_start(dwt[:], DWT[k])
            else:
                nc.sync.dma_start(dwt[:], DWT[k])
            psvm = ps_g.tile([128, BC], F32, tag="pg")
            for c in range(NCH):
                nc.tensor.matmul(
                    psvm[:DIM, c * CH:(c + 1) * CH],
                    vt_sb[:, k * DIM:(k + 1) * DIM],
                    dwt[:, c * CH:(c + 1) * CH],
                    start=True, stop=True,
                )
            vol = volp.tile([DIM, BC], F32, tag="vol")
            nc.vector.tensor_mul(vol[:], s_S[k][:], psvm[:DIM, :])
            s_S[k + 1] = sp.tile([DIM, BC], F32, tag="sp", name=f"snext{k}")
            nc.vector.scalar_tensor_tensor(
                s_S[k + 1][:], s_S[k][:], float(one_prh[k]), vol[:],
                ALU.mult, ALU.add)
            s_M[k + 1] = roundcpy(s_S[k + 1])
            vol_t[k] = vol

        chain(0)
        pending_pprod = None

        for s in range(ns):
            og = [None]
            gg = gen_g(s_M[s], s, og)
            gv = gen_v(s_M[s], s)
            # l1 of both nets, block order (keeps dependent phases 2 apart)
            next(gg); next(gg); next(gv); next(gv)
            # fill the l1->l2 dependency gap with independent PE work
            if s + 1 < ns:
                chain(s + 1)
            if pending_pprod is not None:
                emit_stoch(pending_pprod, s - 1)
                pending_pprod = None
            for _ in range(NH):
                next(gg); next(gg); next(gv); next(gv)
            for _ in gg:
                pass
            for _ in gv:
                pass

            pprod = gp.tile([DIM, BC], MMDT, tag="pp")
            nc.gpsimd.tensor_mul(pprod[:], og[0][:], vol_t[s][:])
            pending_pprod = pprod

        gv_last = gen_v(s_M[ns], ns)
        next(gv_last)
        if pending_pprod is not None:
            emit_stoch(pending_pprod, ns - 1)
            pending_pprod = None
        for _ in gv_last:
            pass
        nc.sync.dma_start(SFT[:], s_S[ns][:])

        # deferred error: err_s = vnew[s] - (1+r h_s) vold[s] - stoch_s
        err1 = const.tile([NS, BC], F32, tag="err1")
        nc.vector.scalar_tensor_tensor(
            err1[:ns, :], vold[0:ns, :], ers_sb[:ns, 0:1], vnew[0:ns, :],
            ALU.mult, ALU.add)
        err2 = const.tile([NS, BC], F32, tag="err2")
        nc.vector.tensor_sub(err2[:ns, :], err1[:ns, :], stbuf[:ns, :])
        sq = const.tile([NS, BC], MMDT, tag="sq")
        nc.scalar.activation(sq[:ns, :], err2[:ns, :], AF.Square)
        pse = ps_g.tile([128, BC], F32, tag="pg")
        for c in range(NCH):
            nc.tensor.matmul(
                pse[:1, c * CH:(c + 1) * CH],
                ones49[:ns, 0:1],
                sq[:ns, c * CH:(c + 1) * CH],
                start=True, stop=True,
            )
        esb = const.tile([1, BC], F32, tag="esb")
        nc.scalar.copy(esb[:], pse[:1, :])
        nc.sync.dma_start(ERR[:], esb[:])

    nc.finalize()
    return nc


def _prep_host(inputs):
    f32 = np.float32
    t = np.asarray(inputs["timegrid"], f32)
    V = np.asarray(inputs["volatility_mat"], f32)
    S0 = np.asarray(inputs["S0"], f32)
    dW = np.asarray(inputs["dW"], f32)
    Wg_in = np.asarray(inputs["Wg_in"], f32)
    bg_in = np.asarray(inputs["bg_in"], f32)
    Wg_h = np.asarray(inputs["Wg_h"], f32)
    bg_h = np.asarray(inputs["bg_h"], f32)
    Wg_out = np.asarray(inputs["Wg_out"], f32)
    bg_out = np.asarray(inputs["bg_out"], f32)
    Wv_in = np.asarray(inputs["Wv_in"], f32)
    bv_in = np.asarray(inputs["bv_in"], f32)
    Wv_h = np.asarray(inputs["Wv_h"], f32)
    bv_h = np.asarray(inputs["bv_h"], f32)
    Wv_out = np.asarray(inputs["Wv_out"], f32)
    bv_out = np.asarray(inputs["bv_out"], f32)

    dt = (t[1:] - t[:-1]).astype(f32)
    sqdt = np.sqrt(dt).astype(f32)
    one_prh = (f32(1.0) + f32(R) * dt).astype(f32)

    vt_all = np.stack([(sqdt[s] * V).T for s in range(NS)])   # [49,100,100]
    vt_all = np.ascontiguousarray(
        vt_all.transpose(1, 0, 2).reshape(DIM, NS * DIM), f32)

    def split_rows(arr):
        # [256, C] -> [128, MT*C] with m-tile blocks side by side
        return np.ascontiguousarray(np.concatenate(
            [arr[mt * 128:(mt + 1) * 128, :] for mt in range(MT)], axis=1), f32)

    def hidden_blocks(Wh):
        return np.ascontiguousarray(np.concatenate(
            [Wh[l][kt * 128:(kt + 1) * 128, :] for l in range(NH) for kt in range(KT)],
            axis=1), f32)

    def out_blocks(Wo):
        return np.ascontiguousarray(np.concatenate(
            [Wo[kt * 128:(kt + 1) * 128, :] for kt in range(KT)], axis=1), f32)

    shared = {
        "VT": vt_all,
        "WG1": np.ascontiguousarray(Wg_in[1:], f32),
        "WGH": hidden_blocks(Wg_h),
        "WGO": out_blocks(Wg_out),
        "WV1": np.ascontiguousarray(Wv_in[1:], f32),
        "WVH": hidden_blocks(Wv_h),
        "WVO": out_blocks(Wv_out),
        "BG": split_rows((bg_in[None, :] + t[:-1, None] * Wg_in[0][None, :]).T),
        "BV": split_rows((bv_in[None, :] + t[:, None] * Wv_in[0][None, :]).T),
        "BGH": split_rows(bg_h.T),
        "BVH": split_rows(bv_h.T),
        "BGO": np.ascontiguousarray(bg_out[:, None], f32),
        "ERS": np.ascontiguousarray(-one_prh[:, None], f32),
    }

    dWT = dW.transpose(0, 2, 1)  # [49, 100, 8192]
    in_maps = []
    for c in range(NCORES):
        m = dict(shared)
        m["S0T"] = np.ascontiguousarray(S0[c * BC:(c + 1) * BC].T, f32)
        m["DWT"] = np.ascontiguousarray(dWT[:, :, c * BC:(c + 1) * BC], f32)
        in_maps.append(m)

    return in_maps, one_prh, float(bv_out[0])


def run(inputs, trace=False, mm_name=None, ns=NS):
    from concourse.bass_utils import run_bass_kernel_spmd

    mm_name = mm_name or MM_DTYPE_NAME
    in_maps, one_prh, bvo = _prep_host(inputs)

    key = (mm_name, ns, one_prh.tobytes(), bvo)
    if key not in _cache:
        _cache[key] = _build(one_prh, bvo, mm_name, ns)
    nc = _cache[key]

    res = run_bass_kernel_spmd(nc, in_maps, list(range(NCORES)), trace=trace)

    f32 = np.float32
    v_f = np.empty((B, 1), f32)
    S_f = np.empty((B, DIM), f32)
    error = np.empty((B, 1), f32)
    for c in range(NCORES):
        r = res.results[c]
        sl = slice(c * BC, (c + 1) * BC)
        S_f[sl] = r["SFT"].T
        v_f[sl] = r["VF"].T
        error[sl] = r["ERR"].T
    return (v_f, S_f, error), res


def kernel(**inputs):
    (v_f, S_f, error), _ = run(inputs, trace=False)
    return v_f, S_f, error


# revision 20
# speedup vs baseline: 1.0605x; 1.0605x over previous
"""Trainium2 Bass kernel for the BSDE solver (nn_BSDE_solver).

Data-parallel over Monte-Carlo paths: batch B=8192 sharded across 8
NeuronCores (1024 paths/core); MLP params, volatility matrix and
per-step scalars replicated. The 49-step time scan runs fully
unrolled on each core.

Layout: activations are feature-major ([feature, batch]) so every MLP
layer is one matmul group with lhsT = the weight matrix as stored
([in, out]) and rhs = activations^T. The time coordinate of the MLP
inputs is folded into a per-step bias (b + t*W[0,:]) on the host;
sqrt(dt) is folded into the volatility matrix per step on the host.

The two MLP evaluations per step are independent given S; the v-net
evaluation is offset by one step (v-eval(s) computes vOld_s at (t_s,
S_s)) and interleaved with g-MLP(s) at m-tile granularity so the PE
never waits on PSUM->SBUF evacuations.

Matmuls run in float32r (single-pass reduced-precision fp32, ~tf32):
all matmul operands must be produced by rounding ops, so weights/dW
are cast-DMA'd via gpsimd and activations are written as float32r by
the evacuation instructions. The S state trajectory stays full fp32.
"""

import numpy as np

# problem constants (hardcoded per spec)
B, DIM, N, W, L = 8192, 100, 50, 256, 4
R = 0.05
NCORES = 8
BC = B // NCORES      # paths per core
CH = 512              # free-dim chunk = one PSUM bank of fp32
NCH = BC // CH
NS = N - 1            # time steps
KT = W // 128         # k-tiles per hidden matmul
MT = W // 128         # m-tiles per hidden layer
NH = L - 1            # hidden layers

MM_DTYPE_NAME = "float32r"

_cache = {}


def _build(one_prh, bvo, mm_name, ns):
    import concourse.bacc as bacc
    import concourse.mybir as mybir
    import concourse.tile as tile
    from contextlib import ExitStack

    F32 = mybir.dt.float32
    MMDT = getattr(mybir.dt, mm_name)
    AF = mybir.ActivationFunctionType
    ALU = mybir.AluOpType
    rounded = MMDT != F32

    nc = bacc.Bacc("TRN2", target_bir_lowering=False, debug=False)

    def din(name, shape):
        return nc.dram_tensor(name, shape, F32, kind="ExternalInput").ap()

    def dout(name, shape):
        return nc.dram_tensor(name, shape, F32, kind="ExternalOutput").ap()

    S0T = din("S0T", [DIM, BC])
    DWT = din("DWT", [NS, DIM, BC])
    VT = din("VT", [DIM, NS * DIM])
    WG1 = din("WG1", [DIM, W])
    WGH = din("WGH", [128, NH * KT * W])
    WGO = din("WGO", [128, KT * DIM])
    WV1 = din("WV1", [DIM, W])
    WVH = din("WVH", [128, NH * KT * W])
    WVO = din("WVO", [128, KT * 1])
    BG = din("BG", [128, MT * NS])
    BV = din("BV", [128, MT * (NS + 1)])
    BGH = din("BGH", [128, MT * NH])
    BVH = din("BVH", [128, MT * NH])
    BGO = din("BGO", [DIM, 1])
    ERS = din("ERS", [NS, 1])
    SFT = dout("SFT", [DIM, BC])
    VF = dout("VF", [1, BC])
    ERR = dout("ERR", [1, BC])

    with tile.TileContext(nc) as tc, ExitStack() as ctx:
        const = ctx.enter_context(tc.tile_pool(name="const", bufs=1))
        dwp = ctx.enter_context(tc.tile_pool(name="dw", bufs=3))
        sp = ctx.enter_context(tc.tile_pool(name="sp", bufs=3))
        smmp = ctx.enter_context(tc.tile_pool(name="smm", bufs=3))
        hp = ctx.enter_context(tc.tile_pool(name="hp", bufs=5))
        gp = ctx.enter_context(tc.tile_pool(name="gp", bufs=2))
        volp = ctx.enter_context(tc.tile_pool(name="volp", bufs=3))
        ps_g = ctx.enter_context(tc.tile_pool(name="psg", bufs=2, space="PSUM"))
        ps_v = ctx.enter_context(tc.tile_pool(name="psv", bufs=2, space="PSUM"))

        def ctile(shape, src, tag, dt=F32):
            t = const.tile(shape, dt, tag=tag)
            if dt == F32:
                nc.sync.dma_start(t[:], src)
            else:
                nc.gpsimd.dma_start(t[:], src)
            return t

        vt_sb = const.tile([DIM, NS * DIM], MMDT, tag="vt")
        nc.gpsimd.dma_start(vt_sb[:, 0:DIM], VT[:, 0:DIM])
        wv1_sb = ctile([DIM, W], WV1[:], "wv1", MMDT)
        wg1_sb = ctile([DIM, W], WG1[:], "wg1", MMDT)
        wvh_sb = ctile([128, NH * KT * W], WVH[:], "wvh", MMDT)
        wgh_sb = ctile([128, NH * KT * W], WGH[:], "wgh", MMDT)
        wvo_sb = ctile([128, KT * 1], WVO[:], "wvo", MMDT)
        wgo_sb = ctile([128, KT * DIM], WGO[:], "wgo", MMDT)
        nc.gpsimd.dma_start(vt_sb[:, DIM:], VT[:, DIM:])
        bg_sb = ctile([128, MT * NS], BG[:], "bg")
        bv_sb = ctile([128, MT * (NS + 1)], BV[:], "bv")
        bgh_sb = ctile([128, MT * NH], BGH[:], "bgh")
        bvh_sb = ctile([128, MT * NH], BVH[:], "bvh")
        bgo_sb = ctile([DIM, 1], BGO[:], "bgo")
        ers_sb = ctile([NS, 1], ERS[:], "ers")

        ones_f = const.tile([DIM, 1], F32, tag="ones_f")
        nc.gpsimd.memset(ones_f[:], 1.0)
        ones49_f = const.tile([NS, 1], F32, tag="o49_f")
        nc.gpsimd.memset(ones49_f[:], 1.0)
        if rounded:
            ones_sb = const.tile([DIM, 1], MMDT, tag="ones")
            nc.gpsimd.tensor_copy(ones_sb[:], ones_f[:])
            ones49 = const.tile([NS, 1], MMDT, tag="o49")
            nc.gpsimd.tensor_copy(ones49[:], ones49_f[:])
        else:
            ones_sb, ones49 = ones_f, ones49_f

        vold = const.tile([NS, BC], F32, tag="vold")
        vnew = const.tile([NS, BC], F32, tag="vnew")
        stbuf = const.tile([NS, BC], F32, tag="stbuf")

        def evac(dst, src, bias, relu, eng):
            if eng == "act":
                if relu:
                    nc.scalar.activation(dst, src, AF.Relu, bias=bias)
                elif isinstance(bias, float):
                    nc.scalar.activation(dst, src, AF.Copy, bias=bias)
                else:
                    nc.scalar.activation(dst, src, AF.Identity, bias=bias)
            else:
                if relu:
                    nc.vector.tensor_scalar(dst, src, bias, 0.0, ALU.add, ALU.max)
                else:
                    nc.vector.tensor_scalar(dst, src, bias, None, ALU.add)

        ENG = {0: "dve", 1: "act"}

        def mlp_core_gen(x_mm, w1_sb, wh_sb, b1col, b1_sb, b1_ncols, bh_sb,
                         pool, ptag, out):
            """Yields after each (layer, m-tile) phase; leaves the final
            hidden tiles in out[0:2]."""
            h_prev = [None, None]
            for mt in range(MT):
                ps = pool.tile([128, BC], F32, tag=ptag)
                for c in range(NCH):
                    nc.tensor.matmul(
                        ps[:, c * CH:(c + 1) * CH],
                        w1_sb[:, mt * 128:(mt + 1) * 128],
                        x_mm[:, c * CH:(c + 1) * CH],
                        start=True, stop=True,
                    )
                h = hp.tile([128, BC], MMDT, tag=ptag + "h")
                evac(h[:], ps[:],
                     b1_sb[:, mt * b1_ncols + b1col:mt * b1_ncols + b1col + 1],
                     True, ENG[mt])
                h_prev[mt] = h
                yield
            for l in range(NH):
                h_new = [None, None]
                for mt in range(MT):
                    ps = pool.tile([128, BC], F32, tag=ptag)
                    for kt in range(KT):
                        wsl = wh_sb[:, (l * KT + kt) * W + mt * 128:
                                    (l * KT + kt) * W + (mt + 1) * 128]
                        for c in range(NCH):
                            nc.tensor.matmul(
                                ps[:, c * CH:(c + 1) * CH],
                                wsl,
                                h_prev[kt][:, c * CH:(c + 1) * CH],
                                start=(kt == 0), stop=(kt == KT - 1),
                            )
                    h = hp.tile([128, BC], MMDT, tag=ptag + "h")
                    evac(h[:], ps[:], bh_sb[:, mt * NH + l:mt * NH + l + 1],
                         True, ENG[mt])
                    h_new[mt] = h
                    yield
                h_prev = h_new
            out[0], out[1] = h_prev

        def gen_g(x_mm, s, out_grad):
            h = [None, None]
            yield from mlp_core_gen(x_mm, wg1_sb, wgh_sb, s, bg_sb, NS,
                                    bgh_sb, ps_g, "pg", h)
            ps = ps_g.tile([128, BC], F32, tag="pg")
            for kt in range(KT):
                for c in range(NCH):
                    nc.tensor.matmul(
                        ps[:DIM, c * CH:(c + 1) * CH],
                        wgo_sb[:, kt * DIM:(kt + 1) * DIM],
                        h[kt][:, c * CH:(c + 1) * CH],
                        start=(kt == 0), stop=(kt == KT - 1),
                    )
            grad = gp.tile([DIM, BC], F32, tag="grad")
            evac(grad[:], ps[:DIM, :], bgo_sb[:, 0:1], False, "act")
            out_grad[0] = grad
            yield

        def gen_v(x_mm, s):
            """v-eval at (t_s, S_s): writes vold[s], vnew[s-1], VF at the end."""
            h = [None, None]
            yield from mlp_core_gen(x_mm, wv1_sb, wvh_sb, s, bv_sb, NS + 1,
                                    bvh_sb, ps_v, "pv", h)
            ps = ps_v.tile([128, BC], F32, tag="pv")
            for kt in range(KT):
                for c in range(NCH):
                    nc.tensor.matmul(
                        ps[:1, c * CH:(c + 1) * CH],
                        wvo_sb[:, kt:kt + 1],
                        h[kt][:, c * CH:(c + 1) * CH],
                        start=(kt == 0), stop=(kt == KT - 1),
                    )
            vtmp = gp.tile([1, BC], F32, tag="vtmp")
            evac(vtmp[:], ps[:1, :], float(bvo), False, "act")
            if s <= ns - 1:
                nc.sync.dma_start(vold[s:s + 1, :], vtmp[:])
            if s >= 1:
                nc.sync.dma_start(vnew[s - 1:s, :], vtmp[:])
            if s == ns:
                nc.sync.dma_start(VF[:], vtmp[:])
            yield

        def roundcpy(t):
            if not rounded:
                return t
            r = smmp.tile([DIM, BC], MMDT, tag="smm")
            nc.gpsimd.dma_start(r[:], t[:])
            return r

        def emit_stoch(pprod, s):
            pss = ps_g.tile([128, BC], F32, tag="pg")
            for c in range(NCH):
                nc.tensor.matmul(
                    pss[:1, c * CH:(c + 1) * CH],
                    ones_sb[:, 0:1],
                    pprod[:, c * CH:(c + 1) * CH],
                    start=True, stop=True,
                )
            stmp = gp.tile([1, BC], F32, tag="stmp")
            evac(stmp[:], pss[:1, :], 0.0, False, "act")
            nc.sync.dma_start(stbuf[s:s + 1, :], stmp[:])

        s_S = [None] * (ns + 1)   # S_k full-precision state tiles
        s_M = [None] * (ns + 1)   # rounded copies for matmul inputs
        vol_t = [None] * ns

        s_S[0] = sp.tile([DIM, BC], F32, tag="sp", name="s0")
        nc.sync.dma_start(s_S[0][:], S0T[:])
        s_M[0] = roundcpy(s_S[0])

        def chain(k):
            # S_k -> S_{k+1}: dW load, vol matmul, vol product, state update
            dwt = dwp.tile([DIM, BC], MMDT, tag="dw")
            if rounded:
                nc.gpsimd.dma_start(dwt[:], DWT[k])
            else:
                nc.sync.dma_start(dwt[:], DWT[k])
            psvm = ps_g.tile([128, BC], F32, tag="pg")
            for c in range(NCH):
                nc.tensor.matmul(
                    psvm[:DIM, c * CH:(c + 1) * CH],
                    vt_sb[:, k * DIM:(k + 1) * DIM],
                    dwt[:, c * CH:(c + 1) * CH],
                    start=True, stop=True,
                )
            vol = volp.tile([DIM, BC], F32, tag="vol")
            nc.vector.tensor_mul(vol[:], s_S[k][:], psvm[:DIM, :])
            s_S[k + 1] = sp.tile([DIM, BC], F32, tag="sp", name=f"snext{k}")
            nc.vector.scalar_tensor_tensor(
                s_S[k + 1][:], s_S[k][:], float(one_prh[k]), vol[:],
                ALU.mult, ALU.add)
            s_M[k + 1] = roundcpy(s_S[k + 1])
            vol_t[k] = vol

        chain(0)
        pending_pprod = None

        for s in range(ns):
            og = [None]
            gg = gen_g(s_M[s], s, og)
            gv = gen_v(s_M[s], s)
            # l1 of both nets, block order (keeps dependent phases 2 apart)
            next(gg); next(gg); next(gv); next(gv)
            # fill the l1->l2 dependency gap with independent PE work
            if s + 1 < ns:
                chain(s + 1)
            if pending_pprod is not None:
                emit_stoch(pending_pprod, s - 1)
                pending_pprod = None
            for _ in range(NH):
                next(gg); next(gg); next(gv); next(gv)
            for _ in gg:
                pass
            for _ in gv:
                pass

            pprod = gp.tile([DIM, BC], MMDT, tag="pp")
            nc.gpsimd.tensor_mul(pprod[:], og[0][:], vol_t[s][:])
            pending_pprod = pprod

        gv_last = gen_v(s_M[ns], ns)
        next(gv_last)
        if pending_pprod is not None:
            emit_stoch(pending_pprod, ns - 1)
            pending_pprod = None
        for _ in gv_last:
            pass
        nc.sync.dma_start(SFT[:], s_S[ns][:])

        # deferred error: err_s = vnew[s] - (1+r h_s) vold[s] - stoch_s
        err1 = const.tile([NS, BC], F32, tag="err1")
        nc.vector.scalar_tensor_tensor(
            err1[:ns, :], vold[0:ns, :], ers_sb[:ns, 0:1], vnew[0:ns, :],
            ALU.mult, ALU.add)
        err2 = const.tile([NS, BC], F32, tag="err2")
        nc.vector.tensor_sub(err2[:ns, :], err1[:ns, :], stbuf[:ns, :])
        sq = const.tile([NS, BC], MMDT, tag="sq")
        nc.scalar.activation(sq[:ns, :], err2[:ns, :], AF.Square)
        pse = ps_g.tile([128, BC], F32, tag="pg")
        for c in range(NCH):
            nc.tensor.matmul(
                pse[:1, c * CH:(c + 1) * CH],
                ones49[:ns, 0:1],
                sq[:ns, c * CH:(c + 1) * CH],
                start=True, stop=True,
            )
        esb = const.tile([1, BC], F32, tag="esb")
        nc.scalar.copy(esb[:], pse[:1, :])
        nc.sync.dma_start(ERR[:], esb[:])

    nc.finalize()
    return nc


def _prep_host(inputs):
    f32 = np.float32
    t = np.asarray(inputs["timegrid"], f32)
    V = np.asarray(inputs["volatility_mat"], f32)
    S0 = np.asarray(inputs["S0"], f32)
    dW = np.asarray(inputs["dW"], f32)
    Wg_in = np.asarray(inputs["Wg_in"], f32)
    bg_in = np.asarray(inputs["bg_in"], f32)
    Wg_h = np.asarray(inputs["Wg_h"], f32)
    bg_h = np.asarray(inputs["bg_h"], f32)
    Wg_out = np.asarray(inputs["Wg_out"], f32)
    bg_out = np.asarray(inputs["bg_out"], f32)
    Wv_in = np.asarray(inputs["Wv_in"], f32)
    bv_in = np.asarray(inputs["bv_in"], f32)
    Wv_h = np.asarray(inputs["Wv_h"], f32)
    bv_h = np.asarray(inputs["bv_h"], f32)
    Wv_out = np.asarray(inputs["Wv_out"], f32)
    bv_out = np.asarray(inputs["bv_out"], f32)

    dt = (t[1:] - t[:-1]).astype(f32)
    sqdt = np.sqrt(dt).astype(f32)
    one_prh = (f32(1.0) + f32(R) * dt).astype(f32)

    vt_all = np.stack([(sqdt[s] * V).T for s in range(NS)])   # [49,100,100]
    vt_all = np.ascontiguousarray(
        vt_all.transpose(1, 0, 2).reshape(DIM, NS * DIM), f32)

    def split_rows(arr):
        # [256, C] -> [128, MT*C] with m-tile blocks side by side
        return np.ascontiguousarray(np.concatenate(
            [arr[mt * 128:(mt + 1) * 128, :] for mt in range(MT)], axis=1), f32)

    def hidden_blocks(Wh):
        return np.ascontiguousarray(np.concatenate(
            [Wh[l][kt * 128:(kt + 1) * 128, :] for l in range(NH) for kt in range(KT)],
            axis=1), f32)

    def out_blocks(Wo):
        return np.ascontiguousarray(np.concatenate(
            [Wo[kt * 128:(kt + 1) * 128, :] for kt in range(KT)], axis=1), f32)

    shared = {
        "VT": vt_all,
        "WG1": np.ascontiguousarray(Wg_in[1:], f32),
        "WGH": hidden_blocks(Wg_h),
        "WGO": out_blocks(Wg_out),
        "WV1": np.ascontiguousarray(Wv_in[1:], f32),
        "WVH": hidden_blocks(Wv_h),
        "WVO": out_blocks(Wv_out),
        "BG": split_rows((bg_in[None, :] + t[:-1, None] * Wg_in[0][None, :]).T),
        "BV": split_rows((bv_in[None, :] + t[:, None] * Wv_in[0][None, :]).T),
        "BGH": split_rows(bg_h.T),
        "BVH": split_rows(bv_h.T),
        "BGO": np.ascontiguousarray(bg_out[:, None], f32),
        "ERS": np.ascontiguousarray(-one_prh[:, None], f32),
    }

    dWT = dW.transpose(0, 2, 1)  # [49, 100, 8192]
    in_maps = []
    for c in range(NCORES):
        m = dict(shared)
        m["S0T"] = np.ascontiguousarray(S0[c * BC:(c + 1) * BC].T, f32)
        m["DWT"] = np.ascontiguousarray(dWT[:, :, c * BC:(c + 1) * BC], f32)
        in_maps.append(m)

    return in_maps, one_prh, float(bv_out[0])


def run(inputs, trace=False, mm_name=None, ns=NS):
    from concourse.bass_utils import run_bass_kernel_spmd

    mm_name = mm_name or MM_DTYPE_NAME
    in_maps, one_prh, bvo = _prep_host(inputs)

    key = (mm_name, ns, one_prh.tobytes(), bvo)
    if key not in _cache:
        _cache[key] = _build(one_prh, bvo, mm_name, ns)
    nc = _cache[key]

    res = run_bass_kernel_spmd(nc, in_maps, list(range(NCORES)), trace=trace)

    f32 = np.float32
    v_f = np.empty((B, 1), f32)
    S_f = np.empty((B, DIM), f32)
    error = np.empty((B, 1), f32)
    for c in range(NCORES):
        r = res.results[c]
        sl = slice(c * BC, (c + 1) * BC)
        S_f[sl] = r["SFT"].T
        v_f[sl] = r["VF"].T
        error[sl] = r["ERR"].T
    return (v_f, S_f, error), res


def kernel(**inputs):
    (v_f, S_f, error), _ = run(inputs, trace=False)
    return v_f, S_f, error


# revision 21
# speedup vs baseline: 1.0706x; 1.0095x over previous
"""Trainium2 Bass kernel for the BSDE solver (nn_BSDE_solver).

Data-parallel over Monte-Carlo paths: batch B=8192 sharded across 8
NeuronCores (1024 paths/core); MLP params, volatility matrix and
per-step scalars replicated. The 49-step time scan runs fully
unrolled on each core.

Layout: activations are feature-major ([feature, batch]) so every MLP
layer is one matmul group with lhsT = the weight matrix as stored
([in, out]) and rhs = activations^T. The time coordinate of the MLP
inputs is folded into a per-step bias (b + t*W[0,:]) on the host;
sqrt(dt) is folded into the volatility matrix per step on the host.

The two MLP evaluations per step are independent given S; the v-net
evaluation is offset by one step (v-eval(s) computes vOld_s at (t_s,
S_s)) and interleaved with g-MLP(s) at m-tile granularity so the PE
never waits on PSUM->SBUF evacuations.

Matmuls run in float32r (single-pass reduced-precision fp32, ~tf32):
all matmul operands must be produced by rounding ops, so weights/dW
are cast-DMA'd via gpsimd and activations are written as float32r by
the evacuation instructions. The S state trajectory stays full fp32.
"""

import numpy as np

# problem constants (hardcoded per spec)
B, DIM, N, W, L = 8192, 100, 50, 256, 4
R = 0.05
NCORES = 8
BC = B // NCORES      # paths per core
CH = 512              # free-dim chunk = one PSUM bank of fp32
NCH = BC // CH
NS = N - 1            # time steps
KT = W // 128         # k-tiles per hidden matmul
MT = W // 128         # m-tiles per hidden layer
NH = L - 1            # hidden layers

MM_DTYPE_NAME = "float32r"

_cache = {}


def _build(one_prh, bvo, mm_name, ns):
    import concourse.bacc as bacc
    import concourse.mybir as mybir
    import concourse.tile as tile
    from contextlib import ExitStack

    F32 = mybir.dt.float32
    MMDT = getattr(mybir.dt, mm_name)
    AF = mybir.ActivationFunctionType
    ALU = mybir.AluOpType
    rounded = MMDT != F32

    nc = bacc.Bacc("TRN2", target_bir_lowering=False, debug=False)

    def din(name, shape):
        return nc.dram_tensor(name, shape, F32, kind="ExternalInput").ap()

    def dout(name, shape):
        return nc.dram_tensor(name, shape, F32, kind="ExternalOutput").ap()

    S0T = din("S0T", [DIM, BC])
    DWT = din("DWT", [NS, DIM, BC])
    VT = din("VT", [DIM, NS * DIM])
    WG1 = din("WG1", [DIM, W])
    WGH = din("WGH", [128, NH * KT * W])
    WGO = din("WGO", [128, KT * DIM])
    WV1 = din("WV1", [DIM, W])
    WVH = din("WVH", [128, NH * KT * W])
    WVO = din("WVO", [128, KT * 1])
    BG = din("BG", [128, MT * NS])
    BV = din("BV", [128, MT * (NS + 1)])
    BGH = din("BGH", [128, MT * NH])
    BVH = din("BVH", [128, MT * NH])
    BGO = din("BGO", [DIM, 1])
    ERS = din("ERS", [NS, 1])
    SFT = dout("SFT", [DIM, BC])
    VF = dout("VF", [1, BC])
    ERR = dout("ERR", [1, BC])

    with tile.TileContext(nc) as tc, ExitStack() as ctx:
        const = ctx.enter_context(tc.tile_pool(name="const", bufs=1))
        dwp = ctx.enter_context(tc.tile_pool(name="dw", bufs=3))
        sp = ctx.enter_context(tc.tile_pool(name="sp", bufs=3))
        smmp = ctx.enter_context(tc.tile_pool(name="smm", bufs=3))
        hp = ctx.enter_context(tc.tile_pool(name="hp", bufs=5))
        gp = ctx.enter_context(tc.tile_pool(name="gp", bufs=2))
        volp = ctx.enter_context(tc.tile_pool(name="volp", bufs=3))
        ps_g = ctx.enter_context(tc.tile_pool(name="psg", bufs=2, space="PSUM"))
        ps_v = ctx.enter_context(tc.tile_pool(name="psv", bufs=2, space="PSUM"))

        def ctile(shape, src, tag, dt=F32):
            t = const.tile(shape, dt, tag=tag)
            if dt == F32:
                nc.sync.dma_start(t[:], src)
            else:
                nc.gpsimd.dma_start(t[:], src)
            return t

        vt_sb = const.tile([DIM, NS * DIM], MMDT, tag="vt")
        nc.gpsimd.dma_start(vt_sb[:, 0:DIM], VT[:, 0:DIM])
        wv1_sb = ctile([DIM, W], WV1[:], "wv1", MMDT)
        wg1_sb = ctile([DIM, W], WG1[:], "wg1", MMDT)
        wvh_sb = ctile([128, NH * KT * W], WVH[:], "wvh", MMDT)
        wgh_sb = ctile([128, NH * KT * W], WGH[:], "wgh", MMDT)
        wvo_sb = ctile([128, KT * 1], WVO[:], "wvo", MMDT)
        wgo_sb = ctile([128, KT * DIM], WGO[:], "wgo", MMDT)
        bg_sb = ctile([128, MT * NS], BG[:], "bg")
        bv_sb = ctile([128, MT * (NS + 1)], BV[:], "bv")
        bgh_sb = ctile([128, MT * NH], BGH[:], "bgh")
        bvh_sb = ctile([128, MT * NH], BVH[:], "bvh")
        bgo_sb = ctile([DIM, 1], BGO[:], "bgo")
        ers_sb = ctile([NS, 1], ERS[:], "ers")

        ones_f = const.tile([DIM, 1], F32, tag="ones_f")
        nc.gpsimd.memset(ones_f[:], 1.0)
        ones49_f = const.tile([NS, 1], F32, tag="o49_f")
        nc.gpsimd.memset(ones49_f[:], 1.0)
        if rounded:
            ones_sb = const.tile([DIM, 1], MMDT, tag="ones")
            nc.gpsimd.tensor_copy(ones_sb[:], ones_f[:])
            ones49 = const.tile([NS, 1], MMDT, tag="o49")
            nc.gpsimd.tensor_copy(ones49[:], ones49_f[:])
        else:
            ones_sb, ones49 = ones_f, ones49_f

        vold = const.tile([NS, BC], F32, tag="vold")
        vnew = const.tile([NS, BC], F32, tag="vnew")
        stbuf = const.tile([NS, BC], F32, tag="stbuf")

        def evac(dst, src, bias, relu, eng):
            if eng == "act":
                if relu:
                    nc.scalar.activation(dst, src, AF.Relu, bias=bias)
                elif isinstance(bias, float):
                    nc.scalar.activation(dst, src, AF.Copy, bias=bias)
                else:
                    nc.scalar.activation(dst, src, AF.Identity, bias=bias)
            else:
                if relu:
                    nc.vector.tensor_scalar(dst, src, bias, 0.0, ALU.add, ALU.max)
                else:
                    nc.vector.tensor_scalar(dst, src, bias, None, ALU.add)

        ENG = {0: "dve", 1: "act"}

        def mlp_core_gen(x_mm, w1_sb, wh_sb, b1col, b1_sb, b1_ncols, bh_sb,
                         pool, ptag, out):
            """Yields after each (layer, m-tile) phase; leaves the final
            hidden tiles in out[0:2]."""
            h_prev = [None, None]
            for mt in range(MT):
                ps = pool.tile([128, BC], F32, tag=ptag)
                for c in range(NCH):
                    nc.tensor.matmul(
                        ps[:, c * CH:(c + 1) * CH],
                        w1_sb[:, mt * 128:(mt + 1) * 128],
                        x_mm[:, c * CH:(c + 1) * CH],
                        start=True, stop=True,
                    )
                h = hp.tile([128, BC], MMDT, tag=ptag + "h")
                evac(h[:], ps[:],
                     b1_sb[:, mt * b1_ncols + b1col:mt * b1_ncols + b1col + 1],
                     True, ENG[mt])
                h_prev[mt] = h
                yield
            for l in range(NH):
                h_new = [None, None]
                for mt in range(MT):
                    ps = pool.tile([128, BC], F32, tag=ptag)
                    for kt in range(KT):
                        wsl = wh_sb[:, (l * KT + kt) * W + mt * 128:
                                    (l * KT + kt) * W + (mt + 1) * 128]
                        for c in range(NCH):
                            nc.tensor.matmul(
                                ps[:, c * CH:(c + 1) * CH],
                                wsl,
                                h_prev[kt][:, c * CH:(c + 1) * CH],
                                start=(kt == 0), stop=(kt == KT - 1),
                            )
                    h = hp.tile([128, BC], MMDT, tag=ptag + "h")
                    evac(h[:], ps[:], bh_sb[:, mt * NH + l:mt * NH + l + 1],
                         True, ENG[mt])
                    h_new[mt] = h
                    yield
                h_prev = h_new
            out[0], out[1] = h_prev

        def gen_g(x_mm, s, out_grad):
            h = [None, None]
            yield from mlp_core_gen(x_mm, wg1_sb, wgh_sb, s, bg_sb, NS,
                                    bgh_sb, ps_g, "pg", h)
            ps = ps_g.tile([128, BC], F32, tag="pg")
            for kt in range(KT):
                for c in range(NCH):
                    nc.tensor.matmul(
                        ps[:DIM, c * CH:(c + 1) * CH],
                        wgo_sb[:, kt * DIM:(kt + 1) * DIM],
                        h[kt][:, c * CH:(c + 1) * CH],
                        start=(kt == 0), stop=(kt == KT - 1),
                    )
            grad = gp.tile([DIM, BC], F32, tag="grad")
            evac(grad[:], ps[:DIM, :], bgo_sb[:, 0:1], False, "act")
            out_grad[0] = grad
            yield

        def gen_v(x_mm, s):
            """v-eval at (t_s, S_s): writes vold[s], vnew[s-1], VF at the end."""
            h = [None, None]
            yield from mlp_core_gen(x_mm, wv1_sb, wvh_sb, s, bv_sb, NS + 1,
                                    bvh_sb, ps_v, "pv", h)
            ps = ps_v.tile([128, BC], F32, tag="pv")
            for kt in range(KT):
                for c in range(NCH):
                    nc.tensor.matmul(
                        ps[:1, c * CH:(c + 1) * CH],
                        wvo_sb[:, kt:kt + 1],
                        h[kt][:, c * CH:(c + 1) * CH],
                        start=(kt == 0), stop=(kt == KT - 1),
                    )
            vtmp = gp.tile([1, BC], F32, tag="vtmp")
            evac(vtmp[:], ps[:1, :], float(bvo), False, "act")
            if s <= ns - 1:
                nc.sync.dma_start(vold[s:s + 1, :], vtmp[:])
            if s >= 1:
                nc.sync.dma_start(vnew[s - 1:s, :], vtmp[:])
            if s == ns:
                nc.sync.dma_start(VF[:], vtmp[:])
            yield

        def roundcpy(t):
            if not rounded:
                return t
            r = smmp.tile([DIM, BC], MMDT, tag="smm")
            nc.gpsimd.dma_start(r[:], t[:])
            return r

        def emit_stoch(pprod, s):
            pss = ps_g.tile([128, BC], F32, tag="pg")
            for c in range(NCH):
                nc.tensor.matmul(
                    pss[:1, c * CH:(c + 1) * CH],
                    ones_sb[:, 0:1],
                    pprod[:, c * CH:(c + 1) * CH],
                    start=True, stop=True,
                )
            stmp = gp.tile([1, BC], F32, tag="stmp")
            evac(stmp[:], pss[:1, :], 0.0, False, "act")
            nc.sync.dma_start(stbuf[s:s + 1, :], stmp[:])

        s_S = [None] * (ns + 1)   # S_k full-precision state tiles
        s_M = [None] * (ns + 1)   # rounded copies for matmul inputs
        vol_t = [None] * ns

        s_S[0] = sp.tile([DIM, BC], F32, tag="sp", name="s0")
        nc.sync.dma_start(s_S[0][:], S0T[:])
        s_M[0] = roundcpy(s_S[0])

        def chain(k):
            # S_k -> S_{k+1}: dW load, vol matmul, vol product, state update
            dwt = dwp.tile([DIM, BC], MMDT, tag="dw")
            if rounded:
                nc.gpsimd.dma_start(dwt[:], DWT[k])
            else:
                nc.sync.dma_start(dwt[:], DWT[k])
            psvm = ps_g.tile([128, BC], F32, tag="pg")
            for c in range(NCH):
                nc.tensor.matmul(
                    psvm[:DIM, c * CH:(c + 1) * CH],
                    vt_sb[:, k * DIM:(k + 1) * DIM],
                    dwt[:, c * CH:(c + 1) * CH],
                    start=True, stop=True,
                )
            vol = volp.tile([DIM, BC], F32, tag="vol")
            nc.vector.tensor_mul(vol[:], s_S[k][:], psvm[:DIM, :])
            s_S[k + 1] = sp.tile([DIM, BC], F32, tag="sp", name=f"snext{k}")
            nc.vector.scalar_tensor_tensor(
                s_S[k + 1][:], s_S[k][:], float(one_prh[k]), vol[:],
                ALU.mult, ALU.add)
            s_M[k + 1] = roundcpy(s_S[k + 1])
            vol_t[k] = vol

        chain(0)
        # VT remainder after the bootstrap casts; front chunks first so
        # early steps unblock before the full tensor lands
        nc.gpsimd.dma_start(vt_sb[:, DIM:4 * DIM], VT[:, DIM:4 * DIM])
        nc.gpsimd.dma_start(vt_sb[:, 4 * DIM:16 * DIM], VT[:, 4 * DIM:16 * DIM])
        nc.gpsimd.dma_start(vt_sb[:, 16 * DIM:], VT[:, 16 * DIM:])
        pending_pprod = None

        for s in range(ns):
            og = [None]
            gg = gen_g(s_M[s], s, og)
            gv = gen_v(s_M[s], s)
            # l1 of both nets, block order (keeps dependent phases 2 apart)
            next(gg); next(gg); next(gv); next(gv)
            # fill the l1->l2 dependency gap with independent PE work
            if s + 1 < ns:
                chain(s + 1)
            if pending_pprod is not None:
                emit_stoch(pending_pprod, s - 1)
                pending_pprod = None
            for _ in range(NH):
                next(gg); next(gg); next(gv); next(gv)
            for _ in gg:
                pass
            for _ in gv:
                pass

            pprod = gp.tile([DIM, BC], MMDT, tag="pp")
            nc.gpsimd.tensor_mul(pprod[:], og[0][:], vol_t[s][:])
            pending_pprod = pprod

        gv_last = gen_v(s_M[ns], ns)
        next(gv_last)
        if pending_pprod is not None:
            emit_stoch(pending_pprod, ns - 1)
            pending_pprod = None
        for _ in gv_last:
            pass
        nc.sync.dma_start(SFT[:], s_S[ns][:])

        # deferred error: err_s = vnew[s] - (1+r h_s) vold[s] - stoch_s
        err1 = const.tile([NS, BC], F32, tag="err1")
        nc.vector.scalar_tensor_tensor(
            err1[:ns, :], vold[0:ns, :], ers_sb[:ns, 0:1], vnew[0:ns, :],
            ALU.mult, ALU.add)
        err2 = const.tile([NS, BC], F32, tag="err2")
        nc.vector.tensor_sub(err2[:ns, :], err1[:ns, :], stbuf[:ns, :])
        sq = const.tile([NS, BC], MMDT, tag="sq")
        nc.scalar.activation(sq[:ns, :], err2[:ns, :], AF.Square)
        pse = ps_g.tile([128, BC], F32, tag="pg")
        for c in range(NCH):
            nc.tensor.matmul(
                pse[:1, c * CH:(c + 1) * CH],
                ones49[:ns, 0:1],
                sq[:ns, c * CH:(c + 1) * CH],
                start=True, stop=True,
            )
        esb = const.tile([1, BC], F32, tag="esb")
        nc.scalar.copy(esb[:], pse[:1, :])
        nc.sync.dma_start(ERR[:], esb[:])

    nc.finalize()
    return nc


def _prep_host(inputs):
    f32 = np.float32
    t = np.asarray(inputs["timegrid"], f32)
    V = np.asarray(inputs["volatility_mat"], f32)
    S0 = np.asarray(inputs["S0"], f32)
    dW = np.asarray(inputs["dW"], f32)
    Wg_in = np.asarray(inputs["Wg_in"], f32)
    bg_in = np.asarray(inputs["bg_in"], f32)
    Wg_h = np.asarray(inputs["Wg_h"], f32)
    bg_h = np.asarray(inputs["bg_h"], f32)
    Wg_out = np.asarray(inputs["Wg_out"], f32)
    bg_out = np.asarray(inputs["bg_out"], f32)
    Wv_in = np.asarray(inputs["Wv_in"], f32)
    bv_in = np.asarray(inputs["bv_in"], f32)
    Wv_h = np.asarray(inputs["Wv_h"], f32)
    bv_h = np.asarray(inputs["bv_h"], f32)
    Wv_out = np.asarray(inputs["Wv_out"], f32)
    bv_out = np.asarray(inputs["bv_out"], f32)

    dt = (t[1:] - t[:-1]).astype(f32)
    sqdt = np.sqrt(dt).astype(f32)
    one_prh = (f32(1.0) + f32(R) * dt).astype(f32)

    vt_all = np.stack([(sqdt[s] * V).T for s in range(NS)])   # [49,100,100]
    vt_all = np.ascontiguousarray(
        vt_all.transpose(1, 0, 2).reshape(DIM, NS * DIM), f32)

    def split_rows(arr):
        # [256, C] -> [128, MT*C] with m-tile blocks side by side
        return np.ascontiguousarray(np.concatenate(
            [arr[mt * 128:(mt + 1) * 128, :] for mt in range(MT)], axis=1), f32)

    def hidden_blocks(Wh):
        return np.ascontiguousarray(np.concatenate(
            [Wh[l][kt * 128:(kt + 1) * 128, :] for l in range(NH) for kt in range(KT)],
            axis=1), f32)

    def out_blocks(Wo):
        return np.ascontiguousarray(np.concatenate(
            [Wo[kt * 128:(kt + 1) * 128, :] for kt in range(KT)], axis=1), f32)

    shared = {
        "VT": vt_all,
        "WG1": np.ascontiguousarray(Wg_in[1:], f32),
        "WGH": hidden_blocks(Wg_h),
        "WGO": out_blocks(Wg_out),
        "WV1": np.ascontiguousarray(Wv_in[1:], f32),
        "WVH": hidden_blocks(Wv_h),
        "WVO": out_blocks(Wv_out),
        "BG": split_rows((bg_in[None, :] + t[:-1, None] * Wg_in[0][None, :]).T),
        "BV": split_rows((bv_in[None, :] + t[:, None] * Wv_in[0][None, :]).T),
        "BGH": split_rows(bg_h.T),
        "BVH": split_rows(bv_h.T),
        "BGO": np.ascontiguousarray(bg_out[:, None], f32),
        "ERS": np.ascontiguousarray(-one_prh[:, None], f32),
    }

    dWT = dW.transpose(0, 2, 1)  # [49, 100, 8192]
    in_maps = []
    for c in range(NCORES):
        m = dict(shared)
        m["S0T"] = np.ascontiguousarray(S0[c * BC:(c + 1) * BC].T, f32)
        m["DWT"] = np.ascontiguousarray(dWT[:, :, c * BC:(c + 1) * BC], f32)
        in_maps.append(m)

    return in_maps, one_prh, float(bv_out[0])


def run(inputs, trace=False, mm_name=None, ns=NS):
    from concourse.bass_utils import run_bass_kernel_spmd

    mm_name = mm_name or MM_DTYPE_NAME
    in_maps, one_prh, bvo = _prep_host(inputs)

    key = (mm_name, ns, one_prh.tobytes(), bvo)
    if key not in _cache:
        _cache[key] = _build(one_prh, bvo, mm_name, ns)
    nc = _cache[key]

    res = run_bass_kernel_spmd(nc, in_maps, list(range(NCORES)), trace=trace)

    f32 = np.float32
    v_f = np.empty((B, 1), f32)
    S_f = np.empty((B, DIM), f32)
    error = np.empty((B, 1), f32)
    for c in range(NCORES):
        r = res.results[c]
        sl = slice(c * BC, (c + 1) * BC)
        S_f[sl] = r["SFT"].T
        v_f[sl] = r["VF"].T
        error[sl] = r["ERR"].T
    return (v_f, S_f, error), res


def kernel(**inputs):
    (v_f, S_f, error), _ = run(inputs, trace=False)
    return v_f, S_f, error
